# revision 113
# baseline (speedup 1.0000x reference)
"""Trainium2 Bass kernel for nn_MultiHeadSelfAttentionBlock.

Data-parallel over batch (B=32 -> 4 per core on 8 cores). fp8e4m3
matmul operands where profitable (fp32 PSUM accumulation), bf16
elsewhere; the ls_gamma=1e-5 layer scale leaves enormous tolerance
headroom, and test.py additionally validates the attention math with
ls=1 (rel err ~1e-3, pure fp8/bf16 quantization noise).

  - All weight preprocessing on host (numpy): BN folded to per-channel
    scale/shift; q / k|v-tap / out weights transposed, 2^8-scaled into
    fp8's normal range (compensated at kf/vf extraction, at EXP's
    scale, and in the epilogue scale tensor p_osc), and packed so
    DoubleRow chunk pairs sit at 16-aligned strides; layer scale
    replicated (2^k-scaled for the fp8 o_norm) into the o-column order.
  - Per item: x loaded once (fp32, for the residual); BN on GPSIMD
    writes flat fp8 xn; Scalar copies it into a zero-padded [c, 34x34]
    fp8 buffer (borders pre-zeroed once).  q proj runs fp8 DoubleRow
    over chunk pairs; the 45 merged k|v conv-tap matmuls read strided
    stride-2 windows of the padded buffer directly (no im2col).
  - Logits [p, l] per head in bf16 (the torch .view head-split bug
    resolves to l = 16*c' + 2*t + par, kd = s_lo); both par halves
    share one 2-bank PSUM tile so a single EXP (fp8 out) serves each
    p-tile.  o runs fp8 DoubleRow over the two p-tiles with a ones
    column in V^T producing the softmax denominator for free.
  - Denominators collect in a 4-partition staging tile; reciprocal (+
    scaled layer scale) in two half-batches (after heads 3 and 7, the
    second deferred past the next item's q-proj to avoid queue
    head-of-line blocking), broadcast via a DRAM bounce, normalize to
    fp8 on GPSIMD.  Output projection is fp8 DoubleRow over nv pairs;
    the epilogue STT applies the 2^-k scale and adds the fp32 residual
    while un-permuting the t-major column order to pixel order.
  - Software-pipelined emission: x/BN prefetched one item ahead,
    item b-1's output projection emitted between taps(b) and
    attention(b).
"""

from contextlib import ExitStack

import os

import numpy as np

import concourse.bacc as bacc
import concourse.bass as bass
import concourse.tile as tile
from concourse import mybir
from concourse.masks import make_identity
from concourse.dve_ops import RECIPROCAL_APPROX_FAST, RECIP_APPROX_FAST_CONSTS

F32 = mybir.dt.float32
BF16 = mybir.dt.bfloat16
F8 = mybir.dt.float8e4
DR = mybir.MatmulPerfMode.DoubleRow
ALU = mybir.AluOpType
ACTF = mybir.ActivationFunctionType
LS_EXP = 23                  # o_norm carries ls * 2^23 to stay in fp8 range
OW_EXP = 8                   # out_w scaled by 2^8 to avoid fp8 denormals

B, C, H, W = 32, 640, 32, 32
NH, KD, VD = 8, 64, 64
S = H * W            # 1024
P = 256              # key/value positions (16x16)
EPS = 1e-3
N_CORES = 8
BPC = B // N_CORES   # 4 batch items per core
NCH = C // 128       # 5 channel chunks
PW = 34              # padded image width
PSZ = PW * PW        # 1156
PCH = 1168           # padded chunk stride, 16-aligned for DoubleRow pairs


def _fap(base, free_off, dims):
    """AP with base's partition dim and explicit free dims [[step, count],...]."""
    return bass.AP(tensor=base.tensor, offset=base.offset + free_off,
                   ap=[base.ap[0]] + dims)


def build_nc():
    nc = bacc.Bacc(None, target_bir_lowering=False, debug=False)

    x4 = nc.dram_tensor("x", [BPC, C, H, W], F32, kind="ExternalInput")
    qwT_d = nc.dram_tensor("p_qwT", [128, NCH * 512], F8, kind="ExternalInput")
    wtap_d = nc.dram_tensor("p_wtap", [128, NCH * 9 * 128], F8,
                            kind="ExternalInput")
    owT_d = nc.dram_tensor("p_owT", [128, 4 * C], F8, kind="ExternalInput")
    kvc_d = nc.dram_tensor("p_kvconst", [128, 1], F32, kind="ExternalInput")
    bnio_d = nc.dram_tensor("p_bnio", [128, 2 * NCH], F32, kind="ExternalInput")
    lsr_d = nc.dram_tensor("p_lsrow", [128, 2 * S], BF16, kind="ExternalInput")
    osc_d = nc.dram_tensor("p_osc", [128, 1], F32, kind="ExternalInput")
    out4 = nc.dram_tensor("out", [BPC, C, H, W], F32, kind="ExternalOutput")
    KSTAGE = int(os.environ.get("KSTAGE", "99"))

    with tile.TileContext(nc) as tc, ExitStack() as ctx:
        wp = ctx.enter_context(tc.tile_pool(name="wp", bufs=1))
        # single PSUM pool, tags sized to exactly 8 banks:
        #   mm 2x[128,512]f32 + kvf 1x[128,256]f32 + lg 2x[128,512]f32
        #   + op 3x[65,512]f32
        pp = ctx.enter_context(tc.tile_pool(name="pp", bufs=1, space="PSUM"))
        xin = ctx.enter_context(tc.tile_pool(name="xin", bufs=3 * NCH))
        xnfp = ctx.enter_context(tc.tile_pool(name="xnfp", bufs=2))
        qbp = ctx.enter_context(tc.tile_pool(name="qbp", bufs=2))
        ep = ctx.enter_context(tc.tile_pool(name="ep", bufs=4))
        kvp = ctx.enter_context(tc.tile_pool(name="kvp", bufs=2))
        orp = ctx.enter_context(tc.tile_pool(name="orp", bufs=8))
        onp = ctx.enter_context(tc.tile_pool(name="onp", bufs=4))
        rbcp = ctx.enter_context(tc.tile_pool(name="rbcp", bufs=4))
        dap = ctx.enter_context(tc.tile_pool(name="dap", bufs=2))
        osb = ctx.enter_context(tc.tile_pool(name="osb", bufs=2))
        drp = ctx.enter_context(tc.tile_pool(name="drp", bufs=2, space="DRAM"))

        # ---------------- setup ----------------
        # item 0's critical path first: BN params, x(0) load + BN, and the
        # xnpad[0] border-zero all start before the rest of the setup.
        bnio = wp.tile([128, 2 * NCH], F32, tag="bnio", name="bnio")
        nc.sync.dma_start(out=bnio[:], in_=bnio_d[:, :])
        xnpad = [wp.tile([128, NCH * PCH], F8, tag=f"xnp{i}", name=f"xnp{i}")
                 for i in range(2)]
        nc.gpsimd.memset(xnpad[0][:], 0.0)

        identf = wp.tile([64, 64], F32, tag="identf", name="identf")
        ones1 = wp.tile([128, 1], BF16, tag="ones1", name="ones1")

        qwT = wp.tile([128, NCH * 512], F8, tag="qwT", name="qwT")
        nc.sync.dma_start(out=qwT[:], in_=qwT_d[:, :])
        wtap = wp.tile([128, NCH * 9 * 128], F8, tag="wtap", name="wtap")
        nc.sync.dma_start(out=wtap[:], in_=wtap_d[:, :])
        owT = wp.tile([128, 4 * C], F8, tag="owT", name="owT")
        nc.sync.dma_start(out=owT[:], in_=owT_d[:, :])
        kvc = wp.tile([128, 1], F32, tag="kvc", name="kvc")
        nc.sync.dma_start(out=kvc[:], in_=kvc_d[:, :])
        lsrow = wp.tile([128, 2 * S], BF16, tag="lsrow", name="lsrow")
        nc.sync.dma_start(out=lsrow[:], in_=lsr_d[:, :])
        osc = wp.tile([128, 1], F32, tag="osc", name="osc")
        nc.sync.dma_start(out=osc[:], in_=osc_d[:, :])

        prev = None  # (b, o_norm tiles, x tiles)
        pending_denom = None  # previous item's deferred half-batch

        def emit_load_bn(b):
            """x load + BN (flat fp8), prefetched one item ahead."""
            xts = []
            for ch in range(NCH):
                xt = xin.tile([128, S], F32, tag="xin", name="xin")
                nc.sync.dma_start(
                    out=xt[:],
                    in_=x4[b, 128 * ch:128 * (ch + 1), :, :].rearrange(
                        "c h w -> c (h w)"))
                xts.append(xt)
            xnf = xnfp.tile([128, NCH * S], F8, tag="xnf8", name="xnf8")
            for ch in range(NCH):
                nc.gpsimd.tensor_scalar(
                    out=xnf[:, S * ch:S * (ch + 1)], in0=xts[ch][:],
                    scalar1=bnio[:, ch:ch + 1],
                    scalar2=bnio[:, NCH + ch:NCH + ch + 1],
                    op0=ALU.mult, op1=ALU.add)
            return xts, xnf

        cur_load = emit_load_bn(0)

        # rest of the setup, off item 0's critical path
        make_identity(nc, identf[:])
        nc.gpsimd.memset(ones1[:], 1.0)
        nc.gpsimd.memset(xnpad[1][:], 0.0)
        # denominator staging: head n lives at partition 32*(n%4), column
        # block S*(n//4) (engines only address start partitions 0/32/64/96);
        # filler partitions hold 1.0 so the batched reciprocal stays finite.
        dall_t = [dap.tile([128, 2 * S], F32, tag="dall", name="dall")
                  for _ in range(2)]
        for i in range(2):
            nc.gpsimd.memset(dall_t[i][:], 1.0)

        def emit_outproj(bp, onorm_p, xt_p):
            for ch in range(NCH):
                ot = osb.tile([128, S], F32, tag="outsb", name="outsb")
                for par in range(2):
                    po = pp.tile([128, 512], F32, tag="mm", bufs=2, name="po")
                    for j in range(2):   # DoubleRow over adjacent nv pairs
                        lhsT = bass.AP(
                            tensor=owT.tensor,
                            offset=owT.offset + 2 * C * j + 256 * ch,
                            ap=[owT.ap[0], [128, 2], [1, 128]])
                        rhs = _fap(onorm_p[j][:], 512 * par,
                                   [[S, 2], [1, 512]])
                        nc.tensor.matmul(po[:], lhsT, rhs, perf_mode=DR,
                                         start=(j == 0), stop=(j == 1))
                    # po col i = 64*t + c' -> output s = 16*c' + 2*t + par
                    sap = [[2, 8], [16, 64]]
                    nc.vector.scalar_tensor_tensor(
                        out=_fap(ot[:], par, sap), in0=po[:],
                        scalar=osc[:, 0:1],
                        in1=_fap(xt_p[ch][:], par, sap),
                        op0=ALU.mult, op1=ALU.add)
                nc.sync.dma_start(
                    out=out4[bp, 128 * ch:128 * (ch + 1), :, :].rearrange(
                        "c h w -> c (h w)"),
                    in_=ot[:])

        # ================= per batch item =================
        for b in range(BPC):
            slot = b % 2
            xts, xnf = cur_load
            # pad-copies here (not at prefetch time) so they don't block
            # the previous item's attention work on the scalar queue
            for ch in range(NCH):
                nc.scalar.activation(
                    _fap(xnpad[slot][:], PCH * ch + PW + 1, [[PW, H], [1, W]]),
                    xnf[:, S * ch:S * (ch + 1)], ACTF.Copy)

            # ---- q projection -> qbuf [s%128, 512*t + c] (t-major) ----
            # DoubleRow over chunk pairs (0,1) and (2,3), chunk 4 normal
            qbuf = qbp.tile([128, 8 * 512], BF16, tag="qbuf", name="qbuf")
            for t in range(8):
                qp = pp.tile([128, 512], F32, tag="mm", bufs=2, name="qp")
                for j in range(2):
                    lhsT = bass.AP(tensor=xnf.tensor,
                                   offset=xnf.offset + S * 2 * j + 128 * t,
                                   ap=[xnf.ap[0], [S, 2], [1, 128]])
                    rhs = bass.AP(tensor=qwT.tensor,
                                  offset=qwT.offset + 512 * 2 * j,
                                  ap=[qwT.ap[0], [512, 2], [1, 512]])
                    nc.tensor.matmul(qp[:], lhsT, rhs, perf_mode=DR,
                                     start=(j == 0), stop=False)
                nc.tensor.matmul(qp[:], xnf[:, 4 * S + 128 * t:4 * S + 128 * (t + 1)],
                                 qwT[:, 4 * 512:5 * 512],
                                 start=False, stop=True)
                nc.vector.tensor_copy(qbuf[:, 512 * t:512 * (t + 1)], qp[:])



            if KSTAGE == 1:
                nc.sync.dma_start(
                    out=out4[b, 0:128, :, :].rearrange("c h w -> c (h w)"),
                    in_=qbuf[:, 0:1024].bitcast(F32))
                continue

            # ---- merged k|v conv taps -> kvf PSUM [64kf || 64vf, 256] ----
            # DoubleRow over chunk pairs; weights carry 2^8, undone below
            kvf = pp.tile([128, 256], F32, tag="mm", bufs=2, name="kvf")
            xb = xnpad[slot]
            for ch in range(NCH):
                for t in range(9):
                    dy, dx = t // 3, t % 3
                    nc.tensor.matmul(
                        kvf[:], wtap[:, 128 * (9 * ch + t):128 * (9 * ch + t + 1)],
                        _fap(xb[:], PCH * ch + PW * dy + dx,
                             [[2 * PW, 16], [2, 16]]),
                        start=(ch == 0 and t == 0),
                        stop=(ch == NCH - 1 and t == 8))
            ikv = float(2.0 ** -OW_EXP)
            kfdup = kvp.tile([128, 256], BF16, tag="f_k", name="f_k")
            nc.vector.tensor_scalar(out=kfdup[0:64, :], in0=kvf[0:64, :],
                                    scalar1=ikv, scalar2=kvc[0:64, :],
                                    op0=ALU.mult, op1=ALU.add)
            nc.vector.tensor_scalar(out=kfdup[64:128, :], in0=kvf[0:64, :],
                                    scalar1=ikv, scalar2=kvc[0:64, :],
                                    op0=ALU.mult, op1=ALU.add)
            vf = kvp.tile([64, 256], F32, tag="f_v", name="f_v")
            nc.vector.tensor_scalar(out=vf[:], in0=kvf[64:128, :],
                                    scalar1=ikv, scalar2=kvc[64:128, :],
                                    op0=ALU.mult, op1=ALU.add)

            # V'^T with ones column, fp8; p-tiles at 16-aligned stride 80
            # (DoubleRow requires the pair-dim step % 16 == 0)
            vT8 = kvp.tile([128, 2 * 80], F8, tag="vT8", name="vT8")
            for pt in range(2):
                tp = pp.tile([128, 512], F32, tag="mm", bufs=2, name="tp")
                nc.tensor.transpose(tp[:128, 0:64],
                                    vf[:, 128 * pt:128 * (pt + 1)],
                                    identf[0:64, 0:64])
                nc.scalar.activation(vT8[:, 80 * pt:80 * pt + 64],
                                     tp[:128, 0:64], ACTF.Copy)
                nc.vector.tensor_copy(vT8[:, 80 * pt + 64:80 * pt + 65],
                                      ones1[:])

            if KSTAGE == 2:
                nc.sync.dma_start(
                    out=out4[b, 0:128, 0:4, :].rearrange("c h w -> c (h w)"),
                    in_=kfdup[:, :].bitcast(F32))
                continue

            # item b-1's deferred half-batch: emitted after kfdup/vT so its
            # DMA-waiting normalizes sit at the tail of the vector queue
            if pending_denom is not None:
                pending_denom(1)
                pending_denom = None

            # prefetch next item's x + BN so its DMAs drain during attention
            if b + 1 < BPC:
                cur_load = emit_load_bn(b + 1)

            # ---- attention ----
            dall = dall_t[slot]
            o_resh = [orp.tile([128, S], BF16, tag="oresh", name="oresh")
                      for _ in range(4)]
            rec = dap.tile([128, 2 * S], F32, tag="rec", bufs=1, name="rec")
            dall2 = dap.tile([128, 2 * S], BF16, tag="dall2", name="dall2")
            dscr = drp.tile([NH, S], BF16, tag="dscr", name="dscr")
            o_norm = []
            rbcs = []

            def denom_halfbatch(blk, last=False):
                # reciprocal + layer scale for heads 4*blk..4*blk+3, then
                # bounce to DRAM and normalize the two finished c2 blocks
                cs = slice(S * blk, S * (blk + 1))
                nc.vector._custom_dve(
                    RECIPROCAL_APPROX_FAST, out=rec[:, cs], in0=dall[:, cs],
                    s0=RECIP_APPROX_FAST_CONSTS["s0"],
                    s1=RECIP_APPROX_FAST_CONSTS["s1"],
                    imm2=RECIP_APPROX_FAST_CONSTS["imm2"])
                nc.vector.tensor_tensor(out=dall2[:, cs], in0=rec[:, cs],
                                        in1=lsrow[:, cs], op=ALU.mult)
                for m in range(4):
                    nc.sync.dma_start(
                        out=dscr[4 * blk + m:4 * blk + m + 1, :],
                        in_=dall2[32 * m:32 * m + 1, cs])
                # one fp8 pair-tile per half: cols [0:S] = c2 even, [S:2S] odd
                on = onp.tile([128, 2 * S], F8, tag="onorm", name="onorm")
                for c2 in (2 * blk, 2 * blk + 1):
                    rbc = rbcp.tile([128, S], BF16, tag="rbc", name="rbc")
                    rbcs.append(rbc)
                    nc.sync.dma_start(
                        out=rbc[:],
                        in_=bass.AP(tensor=dscr.tensor,
                                    offset=dscr.offset + S * 2 * c2,
                                    ap=[[S, 2], [0, 64], [1, S]]))
                    # blk0 normalizes run mid-attention and must stay on
                    # GPSIMD: a DMA-waiting op at the head of the vector
                    # queue blocks the o-copies behind it (measured +30us).
                    # The deferred blk1 chain is emitted at the tail of the
                    # next item's vector queue, so it runs on the faster
                    # DVE with nothing to block.
                    eng = nc.vector if (blk == 1 or (last and c2 % 2)) \
                        else nc.gpsimd
                    eng.tensor_tensor(
                        out=on[:, S * (c2 % 2):S * (c2 % 2 + 1)],
                        in0=o_resh[c2][:], in1=rbc[:], op=ALU.mult)
                o_norm.append(on)

            for n in range(NH):
                E8 = ep.tile([128, 2 * S], F8, tag="E", name="E")
                for pt in range(2):
                    # both par halves share one 2-bank lg tile -> one EXP
                    lg = pp.tile([128, 2 * 512], F32, tag="lg", bufs=2,
                                 name="lg")
                    for par in range(2):
                        # head n's q: cols {512*t + 64*n + c'} of qbuf,
                        # t-major enumeration: lg col i = 64*t + c'
                        # (within-head s = l = 16*c' + 2*t + par; the
                        # un-permute happens in the epilogue STT write)
                        rhs = _fap(qbuf[64 * par:64 * (par + 1)], 64 * n,
                                   [[512, 8], [1, 64]])
                        nc.tensor.matmul(
                            lg[:, 512 * par:512 * (par + 1)],
                            kfdup[64 * par:64 * (par + 1),
                                  128 * pt:128 * (pt + 1)],
                            rhs, start=True, stop=True)
                    # qbuf carries 2^8 (fp8-scaled q weights); undo here
                    nc.scalar.activation(
                        E8[:, S * pt:S * (pt + 1)],
                        lg[:], ACTF.Exp, scale=float(2.0 ** -OW_EXP))
                # o via DoubleRow over the two p-tiles: op cols = j' =
                # 64*t + c' (l = 16*c' + 2*t + par)
                for par in range(2):
                    op_t = pp.tile([65, 512], F32, tag="op", bufs=2,
                                   name="op")
                    lhsT = bass.AP(tensor=vT8.tensor, offset=vT8.offset,
                                   ap=[vT8.ap[0], [80, 2], [1, 65]])
                    nc.tensor.matmul(
                        op_t[:], lhsT,
                        _fap(E8[:], 512 * par, [[S, 2], [1, 512]]),
                        perf_mode=DR, start=True, stop=True)
                    doff = S * (n // 4) + 512 * par
                    nc.scalar.activation(
                        dall[32 * (n % 4):32 * (n % 4) + 1, doff:doff + 512],
                        op_t[64:65, :], ACTF.Copy)
                    nc.vector.tensor_copy(
                        o_resh[n // 2][64 * (n % 2):64 * (n % 2) + 64,
                                       512 * par:512 * (par + 1)],
                        op_t[0:64, :])
                if n == 3:
                    denom_halfbatch(0)

            if KSTAGE == 4:
                nc.sync.dma_start(
                    out=out4[b, 0:128, 0:16, :].rearrange("c h w -> c (h w)"),
                    in_=dall[:, 0:512])
                nc.sync.dma_start(
                    out=out4[b, 0:128, 16:32, :].rearrange("c h w -> c (h w)"),
                    in_=dall[:, 1024:1536])
                nc.sync.dma_start(
                    out=out4[b, 128:256, 0:16, :].rearrange("c h w -> c (h w)"),
                    in_=rec[:, 0:512])
                nc.sync.dma_start(
                    out=out4[b, 128:256, 16:32, :].rearrange("c h w -> c (h w)"),
                    in_=rec[:, 1024:1536])
                nc.sync.dma_start(
                    out=out4[b, 256:384, 0:16, :].rearrange("c h w -> c (h w)"),
                    in_=o_resh[1][:, :].bitcast(F32))
                nc.sync.dma_start(
                    out=out4[b, 384:512, 0:32, :].rearrange("c h w -> c (h w)"),
                    in_=dall2[:, :].bitcast(F32))
                nc.sync.dma_start(
                    out=out4[b, 512:640, 0:16, :].rearrange("c h w -> c (h w)"),
                    in_=rbcs[1][:, :].bitcast(F32))
                continue
            if KSTAGE == 3:
                denom_halfbatch(1)
                for j in range(2):
                    nc.sync.dma_start(
                        out=out4[b, 128 * j:128 * (j + 1), 0:16, :].rearrange(
                            "c h w -> c (h w)"),
                        in_=o_norm[j][:, :].bitcast(F32))
                continue
            if b == BPC - 1:
                # nothing left to overlap with: emit immediately
                denom_halfbatch(1, last=True)
            else:
                pending_denom = denom_halfbatch

            # previous item's output projection, emitted after this item's
            # attention: it fills the PE while this item's deferred
            # denominator chain completes
            if prev is not None:
                emit_outproj(*prev)
            prev = (b, o_norm, xts)

        if prev is not None:
            emit_outproj(*prev)

    nc.finalize()
    return nc


def _pack_inputs(inputs):
    """Host-side weight folding: everything that doesn't depend on x."""
    import ml_dtypes

    f32 = lambda n: np.asarray(inputs[n], dtype=np.float32)
    bnf = {}
    for p in ("in", "k", "v"):
        sc = f32(f"{p}_bn_gamma") / np.sqrt(f32(f"{p}_bn_var") + EPS)
        sh = f32(f"{p}_bn_beta") - f32(f"{p}_bn_mean") * sc
        bnf[p] = (sc, sh)

    def tobf(a):
        return np.ascontiguousarray(a.astype(ml_dtypes.bfloat16))

    def tof8(a):
        return np.ascontiguousarray(a.astype(mybir.dt.np(mybir.dt.float8e4)))

    q_w = f32("q_w") * float(2.0 ** OW_EXP)      # fp8-friendly scale
    qwT = np.zeros((128, NCH * 512), np.float32)
    for ch in range(NCH):
        qwT[:, 512 * ch:512 * (ch + 1)] = q_w[:, 128 * ch:128 * (ch + 1)].T

    sck, shk = bnf["k"]
    scv, shv = bnf["v"]
    kw_s = f32("k_w") * sck[None, :] * 0.125      # [64, 640]
    vw_s = f32("v_w") * scv[None, :]
    f8s = float(2.0 ** OW_EXP)                    # fp8-friendly tap scale
    kdw = f32("k_dw_w").reshape(C, 9)
    vdw = f32("v_dw_w").reshape(C, 9)
    wtap = np.zeros((128, NCH * 9 * 128), np.float32)
    for ch in range(NCH):
        cs = slice(128 * ch, 128 * (ch + 1))
        for t in range(9):
            blk = wtap[:, 128 * (9 * ch + t):128 * (9 * ch + t + 1)]
            blk[:, 0:64] = kw_s[:, cs].T * kdw[cs, t][:, None]
            blk[:, 64:128] = vw_s[:, cs].T * vdw[cs, t][:, None]

    kvconst = np.zeros((128, 1), np.float32)
    kvconst[0:64, 0] = (kw_s @ shk)
    kvconst[64:128, 0] = (vw_s @ shv)

    # pair-interleaved layout for DoubleRow: pair j, chunk ch at
    # cols [2*C*j + 256*ch : +128] = nv 2j, [+128 : +256] = nv 2j+1
    out_w = f32("out_w") * float(2.0 ** 8)   # fp8-friendly scale
    owT = np.zeros((128, 4 * C), np.float32)
    for j in range(2):
        for ch in range(NCH):
            base = 2 * C * j + 256 * ch
            owT[:, base:base + 128] = \
                out_w[128 * ch:128 * (ch + 1), 256 * j:256 * j + 128].T
            owT[:, base + 128:base + 256] = \
                out_w[128 * ch:128 * (ch + 1), 256 * j + 128:256 * (j + 1)].T

    sci, shi = bnf["in"]
    bnio = np.zeros((128, 2 * NCH), np.float32)
    for ch in range(NCH):
        bnio[:, ch] = sci[128 * ch:128 * (ch + 1)]
        bnio[:, NCH + ch] = shi[128 * ch:128 * (ch + 1)]

    # scale ls so o_norm (= o * rec * ls * 2^k) sits in fp8e4's normal range;
    # compensated by the runtime epilogue scale p_osc.  Columns follow the
    # x = 512*par + 64*t + c' order (s = 16*c' + 2*t + par).
    ls_raw = f32("ls_gamma")             # [32]
    ls_exp = int(np.floor(np.log2(8.0 / max(float(np.abs(ls_raw).max()), 1e-30))))
    ls = ls_raw * float(2.0 ** ls_exp)
    colx = np.arange(S)
    par, i = colx // 512, colx % 512
    t, cp = i // 64, i % 64
    ls_perm = ls[(16 * cp + 2 * t + par) % 32]
    lsrow = np.zeros((128, 2 * S), np.float32)
    for n in range(NH):
        lsrow[32 * (n % 4), S * (n // 4):S * (n // 4 + 1)] = ls_perm
    osc = np.full((128, 1), 2.0 ** -(ls_exp + OW_EXP), np.float32)

    return {
        "p_qwT": tof8(qwT),
        "p_wtap": tof8(wtap * f8s),
        "p_owT": tof8(owT),
        "p_kvconst": np.ascontiguousarray(kvconst),
        "p_bnio": np.ascontiguousarray(bnio),
        "p_lsrow": tobf(lsrow),
        "p_osc": np.ascontiguousarray(osc),
    }


def make_in_maps(inputs):
    x = np.ascontiguousarray(np.asarray(inputs["x"], dtype=np.float32))
    base = _pack_inputs(inputs)
    in_maps = []
    for c in range(N_CORES):
        m = dict(base)
        m["x"] = x[c * BPC:(c + 1) * BPC]
        in_maps.append(m)
    return in_maps


_NC_CACHE = None


def kernel(**inputs):
    global _NC_CACHE
    from concourse.bass_utils import run_bass_kernel_spmd

    if _NC_CACHE is None:
        _NC_CACHE = build_nc()
    nc = _NC_CACHE

    in_maps = make_in_maps(inputs)
    res = run_bass_kernel_spmd(nc, in_maps, core_ids=list(range(N_CORES)))
    out = np.concatenate([res.results[c]["out"] for c in range(N_CORES)], axis=0)
    return out.astype(np.float32)


# revision 116
# speedup vs baseline: 1.1434x; 1.1434x over previous
"""Trainium2 Bass kernel for nn_MultiHeadSelfAttentionBlock.

Data-parallel over batch (B=32 -> 4 per core on 8 cores). fp8e4m3
matmul operands where profitable (fp32 PSUM accumulation), bf16
elsewhere; the ls_gamma=1e-5 layer scale leaves enormous tolerance
headroom, and test.py additionally validates the attention math with
ls=1 (rel err ~1e-3, pure fp8/bf16 quantization noise).

  - All weight preprocessing on host (numpy): BN folded to per-channel
    scale/shift; q / k|v-tap / out weights transposed, 2^8-scaled into
    fp8's normal range (compensated at kf/vf extraction, at EXP's
    scale, and in the epilogue scale tensor p_osc), and packed so
    DoubleRow chunk pairs sit at 16-aligned strides; layer scale
    replicated (2^k-scaled for the fp8 o_norm) into the o-column order.
  - Per item: x loaded once (fp32, for the residual); BN on GPSIMD
    writes flat fp8 xn; Scalar copies it into a zero-padded [c, 34x34]
    fp8 buffer (borders pre-zeroed once).  q proj runs fp8 DoubleRow
    over chunk pairs; the 45 merged k|v conv-tap matmuls read strided
    stride-2 windows of the padded buffer directly (no im2col).
  - Logits [p, l] per head in bf16 (the torch .view head-split bug
    resolves to l = 16*c' + 2*t + par, kd = s_lo); both par halves
    share one 2-bank PSUM tile so a single EXP (fp8 out) serves each
    p-tile.  o runs fp8 DoubleRow over the two p-tiles with a ones
    column in V^T producing the softmax denominator for free.
  - Denominators collect in a 4-partition staging tile; reciprocal (+
    scaled layer scale) in two half-batches (after heads 3 and 7, the
    second deferred past the next item's q-proj to avoid queue
    head-of-line blocking), broadcast via a DRAM bounce, normalize to
    fp8 on GPSIMD.  Output projection is fp8 DoubleRow over nv pairs;
    the epilogue STT applies the 2^-k scale and adds the fp32 residual
    while un-permuting the t-major column order to pixel order.
  - Software-pipelined emission: x/BN prefetched one item ahead,
    item b-1's output projection emitted between taps(b) and
    attention(b).
"""

from contextlib import ExitStack

import os

import numpy as np

import concourse.bacc as bacc
import concourse.bass as bass
import concourse.tile as tile
from concourse import mybir
from concourse.masks import make_identity
from concourse.dve_ops import RECIPROCAL_APPROX_FAST, RECIP_APPROX_FAST_CONSTS

F32 = mybir.dt.float32
BF16 = mybir.dt.bfloat16
F8 = mybir.dt.float8e4
DR = mybir.MatmulPerfMode.DoubleRow
ALU = mybir.AluOpType
ACTF = mybir.ActivationFunctionType
LS_EXP = 23                  # o_norm carries ls * 2^23 to stay in fp8 range
OW_EXP = 8                   # out_w scaled by 2^8 to avoid fp8 denormals

B, C, H, W = 32, 640, 32, 32
NH, KD, VD = 8, 64, 64
S = H * W            # 1024
P = 256              # key/value positions (16x16)
EPS = 1e-3
N_CORES = 8
BPC = B // N_CORES   # 4 batch items per core
NCH = C // 128       # 5 channel chunks
PW = 34              # padded image width
PSZ = PW * PW        # 1156
PCH = 1168           # padded chunk stride, 16-aligned for DoubleRow pairs


def _fap(base, free_off, dims):
    """AP with base's partition dim and explicit free dims [[step, count],...]."""
    return bass.AP(tensor=base.tensor, offset=base.offset + free_off,
                   ap=[base.ap[0]] + dims)


def build_nc():
    nc = bacc.Bacc(None, target_bir_lowering=False, debug=False)

    x4 = nc.dram_tensor("x", [BPC, C, H, W], F32, kind="ExternalInput")
    qwT_d = nc.dram_tensor("p_qwT", [128, NCH * 512], F8, kind="ExternalInput")
    wtap_d = nc.dram_tensor("p_wtap", [128, NCH * 9 * 128], F8,
                            kind="ExternalInput")
    owT_d = nc.dram_tensor("p_owT", [128, 4 * C], F8, kind="ExternalInput")
    kvc_d = nc.dram_tensor("p_kvconst", [128, 1], F32, kind="ExternalInput")
    bnio_d = nc.dram_tensor("p_bnio", [128, 2 * NCH], F32, kind="ExternalInput")
    lsr_d = nc.dram_tensor("p_lsrow", [128, 2 * S], BF16, kind="ExternalInput")
    osc_d = nc.dram_tensor("p_osc", [128, 1], F32, kind="ExternalInput")
    out4 = nc.dram_tensor("out", [BPC, C, H, W], F32, kind="ExternalOutput")
    KSTAGE = int(os.environ.get("KSTAGE", "99"))

    with tile.TileContext(nc) as tc, ExitStack() as ctx:
        wp = ctx.enter_context(tc.tile_pool(name="wp", bufs=1))
        # single PSUM pool, tags sized to exactly 8 banks:
        #   mm 2x[128,512]f32 + kvf 1x[128,256]f32 + lg 2x[128,512]f32
        #   + op 3x[65,512]f32
        pp = ctx.enter_context(tc.tile_pool(name="pp", bufs=1, space="PSUM"))
        xin = ctx.enter_context(tc.tile_pool(name="xin", bufs=3 * NCH))
        xnfp = ctx.enter_context(tc.tile_pool(name="xnfp", bufs=2))
        qbp = ctx.enter_context(tc.tile_pool(name="qbp", bufs=2))
        ep = ctx.enter_context(tc.tile_pool(name="ep", bufs=4))
        kvp = ctx.enter_context(tc.tile_pool(name="kvp", bufs=2))
        orp = ctx.enter_context(tc.tile_pool(name="orp", bufs=8))
        onp = ctx.enter_context(tc.tile_pool(name="onp", bufs=4))
        rbcp = ctx.enter_context(tc.tile_pool(name="rbcp", bufs=4))
        dap = ctx.enter_context(tc.tile_pool(name="dap", bufs=2))
        osb = ctx.enter_context(tc.tile_pool(name="osb", bufs=2))
        drp = ctx.enter_context(tc.tile_pool(name="drp", bufs=2, space="DRAM"))

        # ---------------- setup ----------------
        # item 0's critical path first: BN params, x(0) load + BN, and the
        # xnpad[0] border-zero all start before the rest of the setup.
        bnio = wp.tile([128, 2 * NCH], F32, tag="bnio", name="bnio")
        nc.sync.dma_start(out=bnio[:], in_=bnio_d[:, :])
        xnpad = [wp.tile([128, NCH * PCH], F8, tag=f"xnp{i}", name=f"xnp{i}")
                 for i in range(2)]
        nc.gpsimd.memset(xnpad[0][:], 0.0)

        identf = wp.tile([64, 64], F32, tag="identf", name="identf")
        ones1 = wp.tile([128, 1], BF16, tag="ones1", name="ones1")

        qwT = wp.tile([128, NCH * 512], F8, tag="qwT", name="qwT")
        nc.sync.dma_start(out=qwT[:], in_=qwT_d[:, :])
        wtap = wp.tile([128, NCH * 9 * 128], F8, tag="wtap", name="wtap")
        nc.sync.dma_start(out=wtap[:], in_=wtap_d[:, :])
        owT = wp.tile([128, 4 * C], F8, tag="owT", name="owT")
        nc.sync.dma_start(out=owT[:], in_=owT_d[:, :])
        kvc = wp.tile([128, 1], F32, tag="kvc", name="kvc")
        nc.sync.dma_start(out=kvc[:], in_=kvc_d[:, :])
        lsrow = wp.tile([128, 2 * S], BF16, tag="lsrow", name="lsrow")
        nc.sync.dma_start(out=lsrow[:], in_=lsr_d[:, :])
        osc = wp.tile([128, 1], F32, tag="osc", name="osc")
        nc.sync.dma_start(out=osc[:], in_=osc_d[:, :])

        prev = None  # (b, o_norm tiles, x tiles)
        pending_denom = None  # previous item's deferred half-batch

        def emit_load_bn(b):
            """x load + BN (flat fp8), prefetched one item ahead."""
            xts = []
            for ch in range(NCH):
                xt = xin.tile([128, S], F32, tag="xin", name="xin")
                nc.sync.dma_start(
                    out=xt[:],
                    in_=x4[b, 128 * ch:128 * (ch + 1), :, :].rearrange(
                        "c h w -> c (h w)"))
                xts.append(xt)
            xnf = xnfp.tile([128, NCH * S], F8, tag="xnf8", name="xnf8")
            for ch in range(NCH):
                nc.gpsimd.tensor_scalar(
                    out=xnf[:, S * ch:S * (ch + 1)], in0=xts[ch][:],
                    scalar1=bnio[:, ch:ch + 1],
                    scalar2=bnio[:, NCH + ch:NCH + ch + 1],
                    op0=ALU.mult, op1=ALU.add)
            return xts, xnf

        cur_load = emit_load_bn(0)

        # rest of the setup, off item 0's critical path
        make_identity(nc, identf[:])
        nc.gpsimd.memset(ones1[:], 1.0)
        nc.gpsimd.memset(xnpad[1][:], 0.0)
        # denominator staging: head n lives at partition 32*(n%4), column
        # block S*(n//4) (engines only address start partitions 0/32/64/96);
        # filler partitions hold 1.0 so the batched reciprocal stays finite.
        dall_t = [dap.tile([128, 2 * S], F32, tag="dall", name="dall")
                  for _ in range(2)]
        for i in range(2):
            nc.gpsimd.memset(dall_t[i][:], 1.0)

        def emit_outproj(bp, onorm_p, xt_p):
            for ch in range(NCH):
                ot = osb.tile([128, S], F32, tag="outsb", name="outsb")
                for par in range(2):
                    po = pp.tile([128, 512], F32, tag="mm", bufs=2, name="po")
                    for j in range(2):   # DoubleRow over adjacent nv pairs
                        lhsT = bass.AP(
                            tensor=owT.tensor,
                            offset=owT.offset + 2 * C * j + 256 * ch,
                            ap=[owT.ap[0], [128, 2], [1, 128]])
                        rhs = _fap(onorm_p[j][:], 512 * par,
                                   [[S, 2], [1, 512]])
                        nc.tensor.matmul(po[:], lhsT, rhs, perf_mode=DR,
                                         start=(j == 0), stop=(j == 1))
                    # po col i = 64*t + c' -> output s = 16*c' + 2*t + par
                    sap = [[2, 8], [16, 64]]
                    nc.vector.scalar_tensor_tensor(
                        out=_fap(ot[:], par, sap), in0=po[:],
                        scalar=osc[:, 0:1],
                        in1=_fap(xt_p[ch][:], par, sap),
                        op0=ALU.mult, op1=ALU.add)
                nc.sync.dma_start(
                    out=out4[bp, 128 * ch:128 * (ch + 1), :, :].rearrange(
                        "c h w -> c (h w)"),
                    in_=ot[:])

        # ================= per batch item =================
        for b in range(BPC):
            slot = b % 2
            xts, xnf = cur_load
            # pad-copies here (not at prefetch time) so they don't block
            # the previous item's attention work on the scalar queue
            for ch in range(NCH):
                nc.scalar.activation(
                    _fap(xnpad[slot][:], PCH * ch + PW + 1, [[PW, H], [1, W]]),
                    xnf[:, S * ch:S * (ch + 1)], ACTF.Copy)

            # ---- q projection -> qbuf [s%128, 512*t + c] (t-major) ----
            # DoubleRow over chunk pairs (0,1) and (2,3), chunk 4 normal
            qbuf = qbp.tile([128, 8 * 512], BF16, tag="qbuf", name="qbuf")
            for t in range(8):
                qp = pp.tile([128, 512], F32, tag="mm", bufs=2, name="qp")
                for j in range(2):
                    lhsT = bass.AP(tensor=xnf.tensor,
                                   offset=xnf.offset + S * 2 * j + 128 * t,
                                   ap=[xnf.ap[0], [S, 2], [1, 128]])
                    rhs = bass.AP(tensor=qwT.tensor,
                                  offset=qwT.offset + 512 * 2 * j,
                                  ap=[qwT.ap[0], [512, 2], [1, 512]])
                    nc.tensor.matmul(qp[:], lhsT, rhs, perf_mode=DR,
                                     start=(j == 0), stop=False)
                nc.tensor.matmul(qp[:], xnf[:, 4 * S + 128 * t:4 * S + 128 * (t + 1)],
                                 qwT[:, 4 * 512:5 * 512],
                                 start=False, stop=True)
                nc.vector.tensor_copy(qbuf[:, 512 * t:512 * (t + 1)], qp[:])

            # item b-1's second denominator half-batch, emitted here so its
            # vector work sits behind this item's qbuf drains in the queue
            if pending_denom is not None:
                pending_denom(1)
                pending_denom = None



            if KSTAGE == 1:
                nc.sync.dma_start(
                    out=out4[b, 0:128, :, :].rearrange("c h w -> c (h w)"),
                    in_=qbuf[:, 0:1024].bitcast(F32))
                continue

            # ---- merged k|v conv taps -> kvf PSUM [64kf || 64vf, 256] ----
            # DoubleRow over chunk pairs; weights carry 2^8, undone below
            kvf = pp.tile([128, 256], F32, tag="mm", bufs=2, name="kvf")
            xb = xnpad[slot]
            for ch in range(NCH):
                for t in range(9):
                    dy, dx = t // 3, t % 3
                    nc.tensor.matmul(
                        kvf[:], wtap[:, 128 * (9 * ch + t):128 * (9 * ch + t + 1)],
                        _fap(xb[:], PCH * ch + PW * dy + dx,
                             [[2 * PW, 16], [2, 16]]),
                        start=(ch == 0 and t == 0),
                        stop=(ch == NCH - 1 and t == 8))
            ikv = float(2.0 ** -OW_EXP)
            kfdup = kvp.tile([128, 256], BF16, tag="f_k", name="f_k")
            nc.vector.tensor_scalar(out=kfdup[0:64, :], in0=kvf[0:64, :],
                                    scalar1=ikv, scalar2=kvc[0:64, :],
                                    op0=ALU.mult, op1=ALU.add)
            nc.vector.tensor_scalar(out=kfdup[64:128, :], in0=kvf[0:64, :],
                                    scalar1=ikv, scalar2=kvc[0:64, :],
                                    op0=ALU.mult, op1=ALU.add)
            vf = kvp.tile([64, 256], F32, tag="f_v", name="f_v")
            nc.vector.tensor_scalar(out=vf[:], in0=kvf[64:128, :],
                                    scalar1=ikv, scalar2=kvc[64:128, :],
                                    op0=ALU.mult, op1=ALU.add)

            # V'^T with ones column, fp8; p-tiles at 16-aligned stride 80
            # (DoubleRow requires the pair-dim step % 16 == 0)
            vT8 = kvp.tile([128, 2 * 80], F8, tag="vT8", name="vT8")
            for pt in range(2):
                tp = pp.tile([128, 512], F32, tag="mm", bufs=2, name="tp")
                nc.tensor.transpose(tp[:128, 0:64],
                                    vf[:, 128 * pt:128 * (pt + 1)],
                                    identf[0:64, 0:64])
                nc.scalar.activation(vT8[:, 80 * pt:80 * pt + 64],
                                     tp[:128, 0:64], ACTF.Copy)
                nc.vector.tensor_copy(vT8[:, 80 * pt + 64:80 * pt + 65],
                                      ones1[:])

            if KSTAGE == 2:
                nc.sync.dma_start(
                    out=out4[b, 0:128, 0:4, :].rearrange("c h w -> c (h w)"),
                    in_=kfdup[:, :].bitcast(F32))
                continue

            # prefetch next item's x + BN so its DMAs drain during attention
            if b + 1 < BPC:
                cur_load = emit_load_bn(b + 1)

            # ---- attention ----
            dall = dall_t[slot]
            o_resh = [orp.tile([128, S], BF16, tag="oresh", name="oresh")
                      for _ in range(4)]
            rec = dap.tile([128, 2 * S], F32, tag="rec", bufs=1, name="rec")
            dall2 = dap.tile([128, 2 * S], BF16, tag="dall2", name="dall2")
            dscr = drp.tile([NH, S], BF16, tag="dscr", name="dscr")
            o_norm = []
            rbcs = []

            def denom_halfbatch(blk, last=False):
                # reciprocal + layer scale for heads 4*blk..4*blk+3, then
                # bounce to DRAM and normalize the two finished c2 blocks
                cs = slice(S * blk, S * (blk + 1))
                nc.vector._custom_dve(
                    RECIPROCAL_APPROX_FAST, out=rec[:, cs], in0=dall[:, cs],
                    s0=RECIP_APPROX_FAST_CONSTS["s0"],
                    s1=RECIP_APPROX_FAST_CONSTS["s1"],
                    imm2=RECIP_APPROX_FAST_CONSTS["imm2"])
                nc.vector.tensor_tensor(out=dall2[:, cs], in0=rec[:, cs],
                                        in1=lsrow[:, cs], op=ALU.mult)
                for m in range(4):
                    nc.sync.dma_start(
                        out=dscr[4 * blk + m:4 * blk + m + 1, :],
                        in_=dall2[32 * m:32 * m + 1, cs])
                # one fp8 pair-tile per half: cols [0:S] = c2 even, [S:2S] odd
                on = onp.tile([128, 2 * S], F8, tag="onorm", name="onorm")
                for c2 in (2 * blk, 2 * blk + 1):
                    rbc = rbcp.tile([128, S], BF16, tag="rbc", name="rbc")
                    rbcs.append(rbc)
                    nc.sync.dma_start(
                        out=rbc[:],
                        in_=bass.AP(tensor=dscr.tensor,
                                    offset=dscr.offset + S * 2 * c2,
                                    ap=[[S, 2], [0, 64], [1, S]]))
                    # normalizes stay on GPSIMD: a DMA-waiting op anywhere
                    # on the vector FIFO blocks the attention o-copies
                    # behind it (measured +30us mid-attention, +36us even
                    # at the pre-attention queue tail).  Only the final
                    # item (nothing queued behind) splits engines.
                    eng = nc.vector if (last and c2 % 2) else nc.gpsimd
                    eng.tensor_tensor(
                        out=on[:, S * (c2 % 2):S * (c2 % 2 + 1)],
                        in0=o_resh[c2][:], in1=rbc[:], op=ALU.mult)
                o_norm.append(on)

            for n in range(NH):
                E8 = ep.tile([128, 2 * S], F8, tag="E", name="E")
                for pt in range(2):
                    # both par halves share one 2-bank lg tile -> one EXP
                    lg = pp.tile([128, 2 * 512], F32, tag="lg", bufs=2,
                                 name="lg")
                    for par in range(2):
                        # head n's q: cols {512*t + 64*n + c'} of qbuf,
                        # t-major enumeration: lg col i = 64*t + c'
                        # (within-head s = l = 16*c' + 2*t + par; the
                        # un-permute happens in the epilogue STT write)
                        rhs = _fap(qbuf[64 * par:64 * (par + 1)], 64 * n,
                                   [[512, 8], [1, 64]])
                        nc.tensor.matmul(
                            lg[:, 512 * par:512 * (par + 1)],
                            kfdup[64 * par:64 * (par + 1),
                                  128 * pt:128 * (pt + 1)],
                            rhs, start=True, stop=True)
                    # qbuf carries 2^8 (fp8-scaled q weights); undo here
                    nc.scalar.activation(
                        E8[:, S * pt:S * (pt + 1)],
                        lg[:], ACTF.Exp, scale=float(2.0 ** -OW_EXP))
                # o via DoubleRow over the two p-tiles: op cols = j' =
                # 64*t + c' (l = 16*c' + 2*t + par)
                for par in range(2):
                    op_t = pp.tile([65, 512], F32, tag="op", bufs=2,
                                   name="op")
                    lhsT = bass.AP(tensor=vT8.tensor, offset=vT8.offset,
                                   ap=[vT8.ap[0], [80, 2], [1, 65]])
                    nc.tensor.matmul(
                        op_t[:], lhsT,
                        _fap(E8[:], 512 * par, [[S, 2], [1, 512]]),
                        perf_mode=DR, start=True, stop=True)
                    doff = S * (n // 4) + 512 * par
                    nc.scalar.activation(
                        dall[32 * (n % 4):32 * (n % 4) + 1, doff:doff + 512],
                        op_t[64:65, :], ACTF.Copy)
                    nc.vector.tensor_copy(
                        o_resh[n // 2][64 * (n % 2):64 * (n % 2) + 64,
                                       512 * par:512 * (par + 1)],
                        op_t[0:64, :])
                if n == 3:
                    denom_halfbatch(0)

            if KSTAGE == 4:
                nc.sync.dma_start(
                    out=out4[b, 0:128, 0:16, :].rearrange("c h w -> c (h w)"),
                    in_=dall[:, 0:512])
                nc.sync.dma_start(
                    out=out4[b, 0:128, 16:32, :].rearrange("c h w -> c (h w)"),
                    in_=dall[:, 1024:1536])
                nc.sync.dma_start(
                    out=out4[b, 128:256, 0:16, :].rearrange("c h w -> c (h w)"),
                    in_=rec[:, 0:512])
                nc.sync.dma_start(
                    out=out4[b, 128:256, 16:32, :].rearrange("c h w -> c (h w)"),
                    in_=rec[:, 1024:1536])
                nc.sync.dma_start(
                    out=out4[b, 256:384, 0:16, :].rearrange("c h w -> c (h w)"),
                    in_=o_resh[1][:, :].bitcast(F32))
                nc.sync.dma_start(
                    out=out4[b, 384:512, 0:32, :].rearrange("c h w -> c (h w)"),
                    in_=dall2[:, :].bitcast(F32))
                nc.sync.dma_start(
                    out=out4[b, 512:640, 0:16, :].rearrange("c h w -> c (h w)"),
                    in_=rbcs[1][:, :].bitcast(F32))
                continue
            if KSTAGE == 3:
                denom_halfbatch(1)
                for j in range(2):
                    nc.sync.dma_start(
                        out=out4[b, 128 * j:128 * (j + 1), 0:16, :].rearrange(
                            "c h w -> c (h w)"),
                        in_=o_norm[j][:, :].bitcast(F32))
                continue
            if b == BPC - 1:
                # nothing left to overlap with: emit immediately
                denom_halfbatch(1, last=True)
            else:
                pending_denom = denom_halfbatch

            # previous item's output projection, emitted after this item's
            # attention: it fills the PE while this item's deferred
            # denominator chain completes
            if prev is not None:
                emit_outproj(*prev)
            prev = (b, o_norm, xts)

        if prev is not None:
            emit_outproj(*prev)

    nc.finalize()
    return nc


def _pack_inputs(inputs):
    """Host-side weight folding: everything that doesn't depend on x."""
    import ml_dtypes

    f32 = lambda n: np.asarray(inputs[n], dtype=np.float32)
    bnf = {}
    for p in ("in", "k", "v"):
        sc = f32(f"{p}_bn_gamma") / np.sqrt(f32(f"{p}_bn_var") + EPS)
        sh = f32(f"{p}_bn_beta") - f32(f"{p}_bn_mean") * sc
        bnf[p] = (sc, sh)

    def tobf(a):
        return np.ascontiguousarray(a.astype(ml_dtypes.bfloat16))

    def tof8(a):
        return np.ascontiguousarray(a.astype(mybir.dt.np(mybir.dt.float8e4)))

    q_w = f32("q_w") * float(2.0 ** OW_EXP)      # fp8-friendly scale
    qwT = np.zeros((128, NCH * 512), np.float32)
    for ch in range(NCH):
        qwT[:, 512 * ch:512 * (ch + 1)] = q_w[:, 128 * ch:128 * (ch + 1)].T

    sck, shk = bnf["k"]
    scv, shv = bnf["v"]
    kw_s = f32("k_w") * sck[None, :] * 0.125      # [64, 640]
    vw_s = f32("v_w") * scv[None, :]
    f8s = float(2.0 ** OW_EXP)                    # fp8-friendly tap scale
    kdw = f32("k_dw_w").reshape(C, 9)
    vdw = f32("v_dw_w").reshape(C, 9)
    wtap = np.zeros((128, NCH * 9 * 128), np.float32)
    for ch in range(NCH):
        cs = slice(128 * ch, 128 * (ch + 1))
        for t in range(9):
            blk = wtap[:, 128 * (9 * ch + t):128 * (9 * ch + t + 1)]
            blk[:, 0:64] = kw_s[:, cs].T * kdw[cs, t][:, None]
            blk[:, 64:128] = vw_s[:, cs].T * vdw[cs, t][:, None]

    kvconst = np.zeros((128, 1), np.float32)
    kvconst[0:64, 0] = (kw_s @ shk)
    kvconst[64:128, 0] = (vw_s @ shv)

    # pair-interleaved layout for DoubleRow: pair j, chunk ch at
    # cols [2*C*j + 256*ch : +128] = nv 2j, [+128 : +256] = nv 2j+1
    out_w = f32("out_w") * float(2.0 ** 8)   # fp8-friendly scale
    owT = np.zeros((128, 4 * C), np.float32)
    for j in range(2):
        for ch in range(NCH):
            base = 2 * C * j + 256 * ch
            owT[:, base:base + 128] = \
                out_w[128 * ch:128 * (ch + 1), 256 * j:256 * j + 128].T
            owT[:, base + 128:base + 256] = \
                out_w[128 * ch:128 * (ch + 1), 256 * j + 128:256 * (j + 1)].T

    sci, shi = bnf["in"]
    bnio = np.zeros((128, 2 * NCH), np.float32)
    for ch in range(NCH):
        bnio[:, ch] = sci[128 * ch:128 * (ch + 1)]
        bnio[:, NCH + ch] = shi[128 * ch:128 * (ch + 1)]

    # scale ls so o_norm (= o * rec * ls * 2^k) sits in fp8e4's normal range;
    # compensated by the runtime epilogue scale p_osc.  Columns follow the
    # x = 512*par + 64*t + c' order (s = 16*c' + 2*t + par).
    ls_raw = f32("ls_gamma")             # [32]
    ls_exp = int(np.floor(np.log2(8.0 / max(float(np.abs(ls_raw).max()), 1e-30))))
    ls = ls_raw * float(2.0 ** ls_exp)
    colx = np.arange(S)
    par, i = colx // 512, colx % 512
    t, cp = i // 64, i % 64
    ls_perm = ls[(16 * cp + 2 * t + par) % 32]
    lsrow = np.zeros((128, 2 * S), np.float32)
    for n in range(NH):
        lsrow[32 * (n % 4), S * (n // 4):S * (n // 4 + 1)] = ls_perm
    osc = np.full((128, 1), 2.0 ** -(ls_exp + OW_EXP), np.float32)

    return {
        "p_qwT": tof8(qwT),
        "p_wtap": tof8(wtap * f8s),
        "p_owT": tof8(owT),
        "p_kvconst": np.ascontiguousarray(kvconst),
        "p_bnio": np.ascontiguousarray(bnio),
        "p_lsrow": tobf(lsrow),
        "p_osc": np.ascontiguousarray(osc),
    }


def make_in_maps(inputs):
    x = np.ascontiguousarray(np.asarray(inputs["x"], dtype=np.float32))
    base = _pack_inputs(inputs)
    in_maps = []
    for c in range(N_CORES):
        m = dict(base)
        m["x"] = x[c * BPC:(c + 1) * BPC]
        in_maps.append(m)
    return in_maps


_NC_CACHE = None


def kernel(**inputs):
    global _NC_CACHE
    from concourse.bass_utils import run_bass_kernel_spmd

    if _NC_CACHE is None:
        _NC_CACHE = build_nc()
    nc = _NC_CACHE

    in_maps = make_in_maps(inputs)
    res = run_bass_kernel_spmd(nc, in_maps, core_ids=list(range(N_CORES)))
    out = np.concatenate([res.results[c]["out"] for c in range(N_CORES)], axis=0)
    return out.astype(np.float32)


# revision 117
# speedup vs baseline: 1.2156x; 1.0631x over previous
"""Trainium2 Bass kernel for nn_MultiHeadSelfAttentionBlock.

Data-parallel over batch (B=32 -> 4 per core on 8 cores). fp8e4m3
matmul operands where profitable (fp32 PSUM accumulation), bf16
elsewhere; the ls_gamma=1e-5 layer scale leaves enormous tolerance
headroom, and test.py additionally validates the attention math with
ls=1 (rel err ~1e-3, pure fp8/bf16 quantization noise).

  - All weight preprocessing on host (numpy): BN folded to per-channel
    scale/shift; q / k|v-tap / out weights transposed, 2^8-scaled into
    fp8's normal range (compensated at kf/vf extraction, at EXP's
    scale, and in the epilogue scale tensor p_osc), and packed so
    DoubleRow chunk pairs sit at 16-aligned strides; layer scale
    replicated (2^k-scaled for the fp8 o_norm) into the o-column order.
  - Per item: x loaded once (fp32, for the residual); BN on GPSIMD
    writes flat fp8 xn; Scalar copies it into a zero-padded [c, 34x34]
    fp8 buffer (borders pre-zeroed once).  q proj runs fp8 DoubleRow
    over chunk pairs; the 45 merged k|v conv-tap matmuls read strided
    stride-2 windows of the padded buffer directly (no im2col).
  - Logits [p, l] per head in bf16 (the torch .view head-split bug
    resolves to l = 16*c' + 2*t + par, kd = s_lo); both par halves
    share one 2-bank PSUM tile so a single EXP (fp8 out) serves each
    p-tile.  o runs fp8 DoubleRow over the two p-tiles with a ones
    column in V^T producing the softmax denominator for free.
  - Denominators collect in a 4-partition staging tile; reciprocal (+
    scaled layer scale) in two half-batches (after heads 3 and 7, the
    second deferred past the next item's q-proj to avoid queue
    head-of-line blocking), broadcast via a DRAM bounce, normalize to
    fp8 on GPSIMD.  Output projection is fp8 DoubleRow over nv pairs;
    the epilogue STT applies the 2^-k scale and adds the fp32 residual
    while un-permuting the t-major column order to pixel order.
  - Software-pipelined emission: x/BN prefetched one item ahead,
    item b-1's output projection emitted between taps(b) and
    attention(b).
"""

from contextlib import ExitStack

import os

import numpy as np

import concourse.bacc as bacc
import concourse.bass as bass
import concourse.tile as tile
from concourse import mybir
from concourse.masks import make_identity
from concourse.dve_ops import RECIPROCAL_APPROX_FAST, RECIP_APPROX_FAST_CONSTS

F32 = mybir.dt.float32
BF16 = mybir.dt.bfloat16
F8 = mybir.dt.float8e4
DR = mybir.MatmulPerfMode.DoubleRow
ALU = mybir.AluOpType
ACTF = mybir.ActivationFunctionType
LS_EXP = 23                  # o_norm carries ls * 2^23 to stay in fp8 range
OW_EXP = 8                   # out_w scaled by 2^8 to avoid fp8 denormals

B, C, H, W = 32, 640, 32, 32
NH, KD, VD = 8, 64, 64
S = H * W            # 1024
P = 256              # key/value positions (16x16)
EPS = 1e-3
N_CORES = 8
BPC = B // N_CORES   # 4 batch items per core
NCH = C // 128       # 5 channel chunks
PW = 34              # padded image width
PSZ = PW * PW        # 1156
PCH = 1168           # padded chunk stride, 16-aligned for DoubleRow pairs


def _fap(base, free_off, dims):
    """AP with base's partition dim and explicit free dims [[step, count],...]."""
    return bass.AP(tensor=base.tensor, offset=base.offset + free_off,
                   ap=[base.ap[0]] + dims)


def build_nc():
    nc = bacc.Bacc(None, target_bir_lowering=False, debug=False)

    x4 = nc.dram_tensor("x", [BPC, C, H, W], F32, kind="ExternalInput")
    qwT_d = nc.dram_tensor("p_qwT", [128, NCH * 512], F8, kind="ExternalInput")
    wtap_d = nc.dram_tensor("p_wtap", [128, NCH * 9 * 128], F8,
                            kind="ExternalInput")
    owT_d = nc.dram_tensor("p_owT", [128, 4 * C], F8, kind="ExternalInput")
    kvc_d = nc.dram_tensor("p_kvconst", [128, 1], F32, kind="ExternalInput")
    bnio_d = nc.dram_tensor("p_bnio", [128, 2 * NCH], F32, kind="ExternalInput")
    lsr_d = nc.dram_tensor("p_lsrow", [128, 2 * S], BF16, kind="ExternalInput")
    osc_d = nc.dram_tensor("p_osc", [128, 1], F32, kind="ExternalInput")
    out4 = nc.dram_tensor("out", [BPC, C, H, W], F32, kind="ExternalOutput")
    KSTAGE = int(os.environ.get("KSTAGE", "99"))

    with tile.TileContext(nc) as tc, ExitStack() as ctx:
        wp = ctx.enter_context(tc.tile_pool(name="wp", bufs=1))
        # single PSUM pool, tags sized to exactly 8 banks:
        #   mm 2x[128,512]f32 + kvf 1x[128,256]f32 + lg 2x[128,512]f32
        #   + op 3x[65,512]f32
        pp = ctx.enter_context(tc.tile_pool(name="pp", bufs=1, space="PSUM"))
        xin = ctx.enter_context(tc.tile_pool(name="xin", bufs=3 * NCH))
        xnfp = ctx.enter_context(tc.tile_pool(name="xnfp", bufs=2))
        qbp = ctx.enter_context(tc.tile_pool(name="qbp", bufs=2))
        ep = ctx.enter_context(tc.tile_pool(name="ep", bufs=4))
        kvp = ctx.enter_context(tc.tile_pool(name="kvp", bufs=2))
        orp = ctx.enter_context(tc.tile_pool(name="orp", bufs=8))
        onp = ctx.enter_context(tc.tile_pool(name="onp", bufs=4))
        rbcp = ctx.enter_context(tc.tile_pool(name="rbcp", bufs=4))
        dap = ctx.enter_context(tc.tile_pool(name="dap", bufs=2))
        osb = ctx.enter_context(tc.tile_pool(name="osb", bufs=2))
        drp = ctx.enter_context(tc.tile_pool(name="drp", bufs=2, space="DRAM"))

        # ---------------- setup ----------------
        # item 0's critical path first: BN params, x(0) load + BN, and the
        # xnpad[0] border-zero all start before the rest of the setup.
        bnio = wp.tile([128, 2 * NCH], F32, tag="bnio", name="bnio")
        nc.sync.dma_start(out=bnio[:], in_=bnio_d[:, :])
        xnpad = [wp.tile([128, NCH * PCH], F8, tag=f"xnp{i}", name=f"xnp{i}")
                 for i in range(2)]
        nc.gpsimd.memset(xnpad[0][:], 0.0)

        identf = wp.tile([64, 64], F32, tag="identf", name="identf")
        ones1 = wp.tile([128, 1], BF16, tag="ones1", name="ones1")

        qwT = wp.tile([128, NCH * 512], F8, tag="qwT", name="qwT")
        nc.sync.dma_start(out=qwT[:], in_=qwT_d[:, :])
        wtap = wp.tile([128, NCH * 9 * 128], F8, tag="wtap", name="wtap")
        nc.sync.dma_start(out=wtap[:], in_=wtap_d[:, :])
        owT = wp.tile([128, 4 * C], F8, tag="owT", name="owT")
        nc.sync.dma_start(out=owT[:], in_=owT_d[:, :])
        kvc = wp.tile([128, 1], F32, tag="kvc", name="kvc")
        nc.sync.dma_start(out=kvc[:], in_=kvc_d[:, :])
        lsrow = wp.tile([128, 2 * S], BF16, tag="lsrow", name="lsrow")
        nc.sync.dma_start(out=lsrow[:], in_=lsr_d[:, :])
        osc = wp.tile([128, 1], F32, tag="osc", name="osc")
        nc.sync.dma_start(out=osc[:], in_=osc_d[:, :])

        prev = None  # (b, o_norm tiles, x tiles)
        pending_denom = None  # previous item's deferred half-batch

        def emit_load_bn(b):
            """x load + BN (flat fp8), prefetched one item ahead."""
            xts = []
            for ch in range(NCH):
                xt = xin.tile([128, S], F32, tag="xin", name="xin")
                nc.sync.dma_start(
                    out=xt[:],
                    in_=x4[b, 128 * ch:128 * (ch + 1), :, :].rearrange(
                        "c h w -> c (h w)"))
                xts.append(xt)
            xnf = xnfp.tile([128, NCH * S], F8, tag="xnf8", name="xnf8")
            for ch in range(NCH):
                nc.gpsimd.tensor_scalar(
                    out=xnf[:, S * ch:S * (ch + 1)], in0=xts[ch][:],
                    scalar1=bnio[:, ch:ch + 1],
                    scalar2=bnio[:, NCH + ch:NCH + ch + 1],
                    op0=ALU.mult, op1=ALU.add)
            return xts, xnf

        cur_load = emit_load_bn(0)

        # rest of the setup, off item 0's critical path
        make_identity(nc, identf[:])
        nc.gpsimd.memset(ones1[:], 1.0)
        nc.gpsimd.memset(xnpad[1][:], 0.0)
        # denominator staging: head n lives at partition 32*(n%4), column
        # block S*(n//4) (engines only address start partitions 0/32/64/96);
        # filler partitions hold 1.0 so the batched reciprocal stays finite.
        dall_t = [dap.tile([128, 2 * S], F32, tag="dall", name="dall")
                  for _ in range(2)]
        for i in range(2):
            nc.gpsimd.memset(dall_t[i][:], 1.0)

        def emit_outproj(bp, onorm_p, xt_p):
            for ch in range(NCH):
                ot = osb.tile([128, S], F32, tag="outsb", name="outsb")
                for par in range(2):
                    po = pp.tile([128, 512], F32, tag="mm", bufs=2, name="po")
                    for j in range(2):   # DoubleRow over adjacent nv pairs
                        lhsT = bass.AP(
                            tensor=owT.tensor,
                            offset=owT.offset + 2 * C * j + 256 * ch,
                            ap=[owT.ap[0], [128, 2], [1, 128]])
                        rhs = _fap(onorm_p[j][:], 512 * par,
                                   [[S, 2], [1, 512]])
                        nc.tensor.matmul(po[:], lhsT, rhs, perf_mode=DR,
                                         start=(j == 0), stop=(j == 1))
                    # po col i = 64*t + c' -> output s = 16*c' + 2*t + par
                    sap = [[2, 8], [16, 64]]
                    nc.vector.scalar_tensor_tensor(
                        out=_fap(ot[:], par, sap), in0=po[:],
                        scalar=osc[:, 0:1],
                        in1=_fap(xt_p[ch][:], par, sap),
                        op0=ALU.mult, op1=ALU.add)
                nc.sync.dma_start(
                    out=out4[bp, 128 * ch:128 * (ch + 1), :, :].rearrange(
                        "c h w -> c (h w)"),
                    in_=ot[:])

        # ================= per batch item =================
        for b in range(BPC):
            slot = b % 2
            xts, xnf = cur_load
            # pad-copies here (not at prefetch time) so they don't block
            # the previous item's attention work on the scalar queue
            for ch in range(NCH):
                nc.scalar.activation(
                    _fap(xnpad[slot][:], PCH * ch + PW + 1, [[PW, H], [1, W]]),
                    xnf[:, S * ch:S * (ch + 1)], ACTF.Copy)

            # ---- q projection -> qbuf [s%128, 512*t + c] (t-major) ----
            # DoubleRow over chunk pairs (0,1) and (2,3), chunk 4 normal
            qbuf = qbp.tile([128, 8 * 512], BF16, tag="qbuf", name="qbuf")
            for t in range(8):
                qp = pp.tile([128, 512], F32, tag="mm", bufs=2, name="qp")
                for j in range(2):
                    lhsT = bass.AP(tensor=xnf.tensor,
                                   offset=xnf.offset + S * 2 * j + 128 * t,
                                   ap=[xnf.ap[0], [S, 2], [1, 128]])
                    rhs = bass.AP(tensor=qwT.tensor,
                                  offset=qwT.offset + 512 * 2 * j,
                                  ap=[qwT.ap[0], [512, 2], [1, 512]])
                    nc.tensor.matmul(qp[:], lhsT, rhs, perf_mode=DR,
                                     start=(j == 0), stop=False)
                nc.tensor.matmul(qp[:], xnf[:, 4 * S + 128 * t:4 * S + 128 * (t + 1)],
                                 qwT[:, 4 * 512:5 * 512],
                                 start=False, stop=True)
                nc.vector.tensor_copy(qbuf[:, 512 * t:512 * (t + 1)], qp[:])

            # item b-1's second denominator half-batch, emitted here so its
            # vector work sits behind this item's qbuf drains in the queue
            if pending_denom is not None:
                pending_denom(1)
                pending_denom = None



            if KSTAGE == 1:
                nc.sync.dma_start(
                    out=out4[b, 0:128, :, :].rearrange("c h w -> c (h w)"),
                    in_=qbuf[:, 0:1024].bitcast(F32))
                continue

            # ---- merged k|v conv taps -> kvf PSUM [64kf || 64vf, 256] ----
            # DoubleRow over chunk pairs; weights carry 2^8, undone below
            kvf = pp.tile([128, 256], F32, tag="mm", bufs=2, name="kvf")
            xb = xnpad[slot]
            for ch in range(NCH):
                for t in range(9):
                    dy, dx = t // 3, t % 3
                    nc.tensor.matmul(
                        kvf[:], wtap[:, 128 * (9 * ch + t):128 * (9 * ch + t + 1)],
                        _fap(xb[:], PCH * ch + PW * dy + dx,
                             [[2 * PW, 16], [2, 16]]),
                        start=(ch == 0 and t == 0),
                        stop=(ch == NCH - 1 and t == 8))
            ikv = float(2.0 ** -OW_EXP)
            kfdup = kvp.tile([128, 256], BF16, tag="f_k", name="f_k")
            nc.vector.tensor_scalar(out=kfdup[0:64, :], in0=kvf[0:64, :],
                                    scalar1=ikv, scalar2=kvc[0:64, :],
                                    op0=ALU.mult, op1=ALU.add)
            nc.vector.tensor_scalar(out=kfdup[64:128, :], in0=kvf[0:64, :],
                                    scalar1=ikv, scalar2=kvc[0:64, :],
                                    op0=ALU.mult, op1=ALU.add)
            vf = kvp.tile([64, 256], F32, tag="f_v", name="f_v")
            nc.vector.tensor_scalar(out=vf[:], in0=kvf[64:128, :],
                                    scalar1=ikv, scalar2=kvc[64:128, :],
                                    op0=ALU.mult, op1=ALU.add)

            # V'^T with ones column, fp8; p-tiles at 16-aligned stride 80
            # (DoubleRow requires the pair-dim step % 16 == 0)
            vT8 = kvp.tile([128, 2 * 80], F8, tag="vT8", name="vT8")
            for pt in range(2):
                tp = pp.tile([128, 512], F32, tag="mm", bufs=2, name="tp")
                nc.tensor.transpose(tp[:128, 0:64],
                                    vf[:, 128 * pt:128 * (pt + 1)],
                                    identf[0:64, 0:64])
                nc.scalar.activation(vT8[:, 80 * pt:80 * pt + 64],
                                     tp[:128, 0:64], ACTF.Copy)
                nc.vector.tensor_copy(vT8[:, 80 * pt + 64:80 * pt + 65],
                                      ones1[:])

            if KSTAGE == 2:
                nc.sync.dma_start(
                    out=out4[b, 0:128, 0:4, :].rearrange("c h w -> c (h w)"),
                    in_=kfdup[:, :].bitcast(F32))
                continue

            # prefetch next item's x + BN so its DMAs drain during attention
            if b + 1 < BPC:
                cur_load = emit_load_bn(b + 1)

            # ---- attention ----
            dall = dall_t[slot]
            o_resh = [orp.tile([128, S], BF16, tag="oresh", name="oresh")
                      for _ in range(4)]
            rec = dap.tile([128, 2 * S], F32, tag="rec", bufs=1, name="rec")
            dall2 = dap.tile([128, 2 * S], F8, tag="dall2", name="dall2")
            dscr = drp.tile([NH, S], F8, tag="dscr", name="dscr")
            o_norm = []
            rbcs = []

            def denom_halfbatch(blk, last=False):
                # reciprocal + layer scale for heads 4*blk..4*blk+3, then
                # bounce to DRAM and normalize the two finished c2 blocks
                cs = slice(S * blk, S * (blk + 1))
                nc.vector._custom_dve(
                    RECIPROCAL_APPROX_FAST, out=rec[:, cs], in0=dall[:, cs],
                    s0=RECIP_APPROX_FAST_CONSTS["s0"],
                    s1=RECIP_APPROX_FAST_CONSTS["s1"],
                    imm2=RECIP_APPROX_FAST_CONSTS["imm2"])
                nc.vector.tensor_tensor(out=dall2[:, cs], in0=rec[:, cs],
                                        in1=lsrow[:, cs], op=ALU.mult)
                for m in range(4):
                    nc.sync.dma_start(
                        out=dscr[4 * blk + m:4 * blk + m + 1, :],
                        in_=dall2[32 * m:32 * m + 1, cs])
                # one fp8 pair-tile per half: cols [0:S] = c2 even, [S:2S] odd
                on = onp.tile([128, 2 * S], F8, tag="onorm", name="onorm")
                for c2 in (2 * blk, 2 * blk + 1):
                    rbc = rbcp.tile([128, S], F8, tag="rbc", name="rbc")
                    rbcs.append(rbc)
                    nc.sync.dma_start(
                        out=rbc[:],
                        in_=bass.AP(tensor=dscr.tensor,
                                    offset=dscr.offset + S * 2 * c2,
                                    ap=[[S, 2], [0, 64], [1, S]]))
                    # normalizes stay on GPSIMD: a DMA-waiting op anywhere
                    # on the vector FIFO blocks the attention o-copies
                    # behind it (measured +30us mid-attention, +36us even
                    # at the pre-attention queue tail).  Only the final
                    # item (nothing queued behind) splits engines.
                    eng = nc.vector if (last and c2 % 2) else nc.gpsimd
                    eng.tensor_tensor(
                        out=on[:, S * (c2 % 2):S * (c2 % 2 + 1)],
                        in0=o_resh[c2][:], in1=rbc[:], op=ALU.mult)
                o_norm.append(on)

            for n in range(NH):
                E8 = ep.tile([128, 2 * S], F8, tag="E", name="E")
                for pt in range(2):
                    # both par halves share one 2-bank lg tile -> one EXP
                    lg = pp.tile([128, 2 * 512], F32, tag="lg", bufs=2,
                                 name="lg")
                    for par in range(2):
                        # head n's q: cols {512*t + 64*n + c'} of qbuf,
                        # t-major enumeration: lg col i = 64*t + c'
                        # (within-head s = l = 16*c' + 2*t + par; the
                        # un-permute happens in the epilogue STT write)
                        rhs = _fap(qbuf[64 * par:64 * (par + 1)], 64 * n,
                                   [[512, 8], [1, 64]])
                        nc.tensor.matmul(
                            lg[:, 512 * par:512 * (par + 1)],
                            kfdup[64 * par:64 * (par + 1),
                                  128 * pt:128 * (pt + 1)],
                            rhs, start=True, stop=True)
                    # qbuf carries 2^8 (fp8-scaled q weights); undo here
                    nc.scalar.activation(
                        E8[:, S * pt:S * (pt + 1)],
                        lg[:], ACTF.Exp, scale=float(2.0 ** -OW_EXP))
                # o via DoubleRow over the two p-tiles: op cols = j' =
                # 64*t + c' (l = 16*c' + 2*t + par)
                for par in range(2):
                    op_t = pp.tile([65, 512], F32, tag="op", bufs=2,
                                   name="op")
                    lhsT = bass.AP(tensor=vT8.tensor, offset=vT8.offset,
                                   ap=[vT8.ap[0], [80, 2], [1, 65]])
                    nc.tensor.matmul(
                        op_t[:], lhsT,
                        _fap(E8[:], 512 * par, [[S, 2], [1, 512]]),
                        perf_mode=DR, start=True, stop=True)
                    doff = S * (n // 4) + 512 * par
                    nc.scalar.activation(
                        dall[32 * (n % 4):32 * (n % 4) + 1, doff:doff + 512],
                        op_t[64:65, :], ACTF.Copy)
                    nc.vector.tensor_copy(
                        o_resh[n // 2][64 * (n % 2):64 * (n % 2) + 64,
                                       512 * par:512 * (par + 1)],
                        op_t[0:64, :])
                if n == 3:
                    denom_halfbatch(0)

            if KSTAGE == 4:
                nc.sync.dma_start(
                    out=out4[b, 0:128, 0:16, :].rearrange("c h w -> c (h w)"),
                    in_=dall[:, 0:512])
                nc.sync.dma_start(
                    out=out4[b, 0:128, 16:32, :].rearrange("c h w -> c (h w)"),
                    in_=dall[:, 1024:1536])
                nc.sync.dma_start(
                    out=out4[b, 128:256, 0:16, :].rearrange("c h w -> c (h w)"),
                    in_=rec[:, 0:512])
                nc.sync.dma_start(
                    out=out4[b, 128:256, 16:32, :].rearrange("c h w -> c (h w)"),
                    in_=rec[:, 1024:1536])
                nc.sync.dma_start(
                    out=out4[b, 256:384, 0:16, :].rearrange("c h w -> c (h w)"),
                    in_=o_resh[1][:, :].bitcast(F32))
                nc.sync.dma_start(
                    out=out4[b, 384:512, 0:32, :].rearrange("c h w -> c (h w)"),
                    in_=dall2[:, :].bitcast(F32))
                nc.sync.dma_start(
                    out=out4[b, 512:640, 0:16, :].rearrange("c h w -> c (h w)"),
                    in_=rbcs[1][:, :].bitcast(F32))
                continue
            if KSTAGE == 3:
                denom_halfbatch(1)
                for j in range(2):
                    nc.sync.dma_start(
                        out=out4[b, 128 * j:128 * (j + 1), 0:16, :].rearrange(
                            "c h w -> c (h w)"),
                        in_=o_norm[j][:, :].bitcast(F32))
                continue
            if b == BPC - 1:
                # nothing left to overlap with: emit immediately
                denom_halfbatch(1, last=True)
            else:
                pending_denom = denom_halfbatch

            # previous item's output projection, emitted after this item's
            # attention: it fills the PE while this item's deferred
            # denominator chain completes
            if prev is not None:
                emit_outproj(*prev)
            prev = (b, o_norm, xts)

        if prev is not None:
            emit_outproj(*prev)

    nc.finalize()
    return nc


def _pack_inputs(inputs):
    """Host-side weight folding: everything that doesn't depend on x."""
    import ml_dtypes

    f32 = lambda n: np.asarray(inputs[n], dtype=np.float32)
    bnf = {}
    for p in ("in", "k", "v"):
        sc = f32(f"{p}_bn_gamma") / np.sqrt(f32(f"{p}_bn_var") + EPS)
        sh = f32(f"{p}_bn_beta") - f32(f"{p}_bn_mean") * sc
        bnf[p] = (sc, sh)

    def tobf(a):
        return np.ascontiguousarray(a.astype(ml_dtypes.bfloat16))

    def tof8(a):
        return np.ascontiguousarray(a.astype(mybir.dt.np(mybir.dt.float8e4)))

    q_w = f32("q_w") * float(2.0 ** OW_EXP)      # fp8-friendly scale
    qwT = np.zeros((128, NCH * 512), np.float32)
    for ch in range(NCH):
        qwT[:, 512 * ch:512 * (ch + 1)] = q_w[:, 128 * ch:128 * (ch + 1)].T

    sck, shk = bnf["k"]
    scv, shv = bnf["v"]
    kw_s = f32("k_w") * sck[None, :] * 0.125      # [64, 640]
    vw_s = f32("v_w") * scv[None, :]
    f8s = float(2.0 ** OW_EXP)                    # fp8-friendly tap scale
    kdw = f32("k_dw_w").reshape(C, 9)
    vdw = f32("v_dw_w").reshape(C, 9)
    wtap = np.zeros((128, NCH * 9 * 128), np.float32)
    for ch in range(NCH):
        cs = slice(128 * ch, 128 * (ch + 1))
        for t in range(9):
            blk = wtap[:, 128 * (9 * ch + t):128 * (9 * ch + t + 1)]
            blk[:, 0:64] = kw_s[:, cs].T * kdw[cs, t][:, None]
            blk[:, 64:128] = vw_s[:, cs].T * vdw[cs, t][:, None]

    kvconst = np.zeros((128, 1), np.float32)
    kvconst[0:64, 0] = (kw_s @ shk)
    kvconst[64:128, 0] = (vw_s @ shv)

    # pair-interleaved layout for DoubleRow: pair j, chunk ch at
    # cols [2*C*j + 256*ch : +128] = nv 2j, [+128 : +256] = nv 2j+1
    out_w = f32("out_w") * float(2.0 ** 8)   # fp8-friendly scale
    owT = np.zeros((128, 4 * C), np.float32)
    for j in range(2):
        for ch in range(NCH):
            base = 2 * C * j + 256 * ch
            owT[:, base:base + 128] = \
                out_w[128 * ch:128 * (ch + 1), 256 * j:256 * j + 128].T
            owT[:, base + 128:base + 256] = \
                out_w[128 * ch:128 * (ch + 1), 256 * j + 128:256 * (j + 1)].T

    sci, shi = bnf["in"]
    bnio = np.zeros((128, 2 * NCH), np.float32)
    for ch in range(NCH):
        bnio[:, ch] = sci[128 * ch:128 * (ch + 1)]
        bnio[:, NCH + ch] = shi[128 * ch:128 * (ch + 1)]

    # scale ls so o_norm (= o * rec * ls * 2^k) sits in fp8e4's normal range;
    # compensated by the runtime epilogue scale p_osc.  Columns follow the
    # x = 512*par + 64*t + c' order (s = 16*c' + 2*t + par).
    ls_raw = f32("ls_gamma")             # [32]
    ls_exp = int(np.floor(np.log2(8.0 / max(float(np.abs(ls_raw).max()), 1e-30))))
    ls = ls_raw * float(2.0 ** ls_exp)
    colx = np.arange(S)
    par, i = colx // 512, colx % 512
    t, cp = i // 64, i % 64
    ls_perm = ls[(16 * cp + 2 * t + par) % 32]
    lsrow = np.zeros((128, 2 * S), np.float32)
    for n in range(NH):
        lsrow[32 * (n % 4), S * (n // 4):S * (n // 4 + 1)] = ls_perm
    osc = np.full((128, 1), 2.0 ** -(ls_exp + OW_EXP), np.float32)

    return {
        "p_qwT": tof8(qwT),
        "p_wtap": tof8(wtap * f8s),
        "p_owT": tof8(owT),
        "p_kvconst": np.ascontiguousarray(kvconst),
        "p_bnio": np.ascontiguousarray(bnio),
        "p_lsrow": tobf(lsrow),
        "p_osc": np.ascontiguousarray(osc),
    }


def make_in_maps(inputs):
    x = np.ascontiguousarray(np.asarray(inputs["x"], dtype=np.float32))
    base = _pack_inputs(inputs)
    in_maps = []
    for c in range(N_CORES):
        m = dict(base)
        m["x"] = x[c * BPC:(c + 1) * BPC]
        in_maps.append(m)
    return in_maps


_NC_CACHE = None


def kernel(**inputs):
    global _NC_CACHE
    from concourse.bass_utils import run_bass_kernel_spmd

    if _NC_CACHE is None:
        _NC_CACHE = build_nc()
    nc = _NC_CACHE

    in_maps = make_in_maps(inputs)
    res = run_bass_kernel_spmd(nc, in_maps, core_ids=list(range(N_CORES)))
    out = np.concatenate([res.results[c]["out"] for c in range(N_CORES)], axis=0)
    return out.astype(np.float32)
